# revision 7
# baseline (speedup 1.0000x reference)
"""Chamfer distance loss kernel for Trainium2 (8 NeuronCores).

Problem: template [4, 8192, 3] f32, source [4, 8192, 3] f32 ->
scalar 0.5*(mean_n sqrt(min_m d2) + mean_m sqrt(min_n d2)) over all batches,
d2 = squared euclidean distance, clamped at 0.

Sharding: core c handles batch b = c//2, template half h = c%2
(4096 template rows x all 8192 source points).

Device algorithm (per core):
  e[n, m] = t.s - 0.5||t||^2 - 0.5||s||^2  = -0.5*d2[n, m]
  computed as a K=13 fp16 split-precision matmul (hi/lo decomposition of
  the coordinates and norms, products accumulated in fp32 PSUM) -- full
  fp32-grade accuracy at 1 cycle/row on the PE.
  Row mins:  d2rowmin = max(-2 * max_m e, 0)  -- DVE tensor_scalar max-accum.
  Col mins:  partial max_n e accumulated elementwise (DVE tensor_tensor max),
  final partition/core reduction + sqrt/mean on host (tiny arrays).
"""

import numpy as np

F16 = np.float16
F32 = np.float32

B, N, M, D = 4, 8192, 8192, 3
N_CORES = 8
NSHARD = N // 2          # template rows per core (4096)
NT = NSHARD // 128       # n-tiles per core (32)
MG = M // 2048           # psum groups per n-tile (4)
K = 13                   # augmented contraction dim

_NC_CACHE = {}


def _build_nc():
    import concourse.bacc as bacc
    import concourse.mybir as mybir
    from concourse.tile import TileContext

    f16 = mybir.dt.float16
    f32 = mybir.dt.float32
    Alu = mybir.AluOpType

    nc = bacc.Bacc()
    lhsT = nc.declare_dram_parameter("lhsT", [K, NSHARD], f16, isOutput=False)
    rhs = nc.declare_dram_parameter("rhs", [K, M], f16, isOutput=False)
    rowmax_o = nc.declare_dram_parameter("rowmax", [128, NT], f32, isOutput=True)
    colmax_o = nc.declare_dram_parameter("colmax", [128, M], f16, isOutput=True)

    with TileContext(nc) as tc:
        with (
            tc.tile_pool(name="const", bufs=1) as cpool,
            tc.tile_pool(name="psum", bufs=2, space="PSUM") as ppool,
            tc.tile_pool(name="ebuf", bufs=2) as epool,
        ):
            lhsT_sb = cpool.tile([K, NSHARD], f16)
            rhs_sb = cpool.tile([K, M], f16)
            nc.gpsimd.dma_start(lhsT_sb[:], lhsT[:])
            nc.gpsimd.dma_start(rhs_sb[:], rhs[:])

            cmax = cpool.tile([128, M], f16)
            rowmax = cpool.tile([128, NT], f32)
            scratch = cpool.tile([128, M], f16)
            nc.vector.memset(cmax[:], -60000.0)

            for ti in range(NT):
                e = epool.tile([128, M], f16, tag="e")
                lw = lhsT_sb[:, ti * 128:(ti + 1) * 128]
                for g in range(MG):
                    ps = ppool.tile([128, 2048], f32, tag="ps")
                    for j in range(4):
                        mj = g * 4 + j
                        nc.tensor.matmul(
                            ps[:, j * 512:(j + 1) * 512],
                            lw,
                            rhs_sb[:, mj * 512:(mj + 1) * 512],
                            start=True,
                            stop=True,
                        )
                    nc.scalar.copy(e[:, g * 2048:(g + 1) * 2048], ps[:])
                # row maxes of this n-tile (free-dim max-reduce, 4x mode)
                nc.vector.tensor_scalar(
                    out=scratch[:],
                    in0=e[:],
                    scalar1=0.0,
                    scalar2=None,
                    op0=Alu.add,
                    op1=Alu.max,
                    accum_out=rowmax[:, ti:ti + 1],
                )
                # col maxes accumulated across n-tiles (2x mode)
                nc.vector.tensor_tensor(cmax[:], cmax[:], e[:], Alu.max)

            nc.gpsimd.dma_start(rowmax_o[:], rowmax[:])
            nc.gpsimd.dma_start(colmax_o[:], cmax[:])
    return nc


def get_nc():
    if "nc" not in _NC_CACHE:
        nc = _build_nc()
        nc.finalize()
        _NC_CACHE["nc"] = nc
    return _NC_CACHE["nc"]


def _split16(x32):
    """Split fp32 array into (hi, lo) fp16 pair with hi + lo ~= x."""
    hi = x32.astype(F16)
    lo = (x32 - hi.astype(F32)).astype(F16)
    return hi, lo


def _build_lhsT(t):
    """t: [n, 3] f32 template shard -> [13, n] f16 stationary operand."""
    n = t.shape[0]
    th, tl = _split16(t)
    t2 = (t * t).sum(axis=1, dtype=F32)
    u = -0.5 * t2
    uh, ul = _split16(u)
    out = np.empty((K, n), dtype=F16)
    out[0:3] = th.T
    out[3:6] = tl.T
    out[6:9] = th.T
    out[9] = uh
    out[10] = ul
    out[11] = 1.0
    out[12] = 1.0
    return out


def _build_rhs(s):
    """s: [m, 3] f32 source -> [13, m] f16 moving operand."""
    m = s.shape[0]
    sh, sl = _split16(s)
    s2 = (s * s).sum(axis=1, dtype=F32)
    v = -0.5 * s2
    vh, vl = _split16(v)
    out = np.empty((K, m), dtype=F16)
    out[0:3] = sh.T
    out[3:6] = sh.T
    out[6:9] = sl.T
    out[9] = 1.0
    out[10] = 1.0
    out[11] = vh
    out[12] = vl
    return out


def make_in_maps(template, source):
    template = np.asarray(template, dtype=F32)
    source = np.asarray(source, dtype=F32)
    in_maps = []
    for c in range(N_CORES):
        b, h = divmod(c, 2)
        t = template[b, h * NSHARD:(h + 1) * NSHARD]
        s = source[b]
        in_maps.append({"lhsT": _build_lhsT(t), "rhs": _build_rhs(s)})
    return in_maps


def finalize(results):
    """results: list of 8 dicts with 'rowmax' [128, NT] f32, 'colmax' [128, M] f16."""
    row_sqrts = []
    for c in range(N_CORES):
        rm = np.asarray(results[c]["rowmax"], dtype=F32)
        row_sqrts.append(np.sqrt(np.maximum(-2.0 * rm, 0.0), dtype=F32).ravel())
    c01 = np.mean(np.concatenate(row_sqrts), dtype=F32)

    col_sqrts = []
    for b in range(B):
        cm0 = np.asarray(results[2 * b]["colmax"])
        cm1 = np.asarray(results[2 * b + 1]["colmax"])
        cm = np.maximum(cm0, cm1).max(axis=0).astype(F32)  # [M]
        col_sqrts.append(np.sqrt(np.maximum(-2.0 * cm, 0.0), dtype=F32))
    c10 = np.mean(np.concatenate(col_sqrts), dtype=F32)
    return np.float32((c01 + c10) * 0.5)


def kernel(template, source):
    from concourse.bass_utils import run_bass_kernel_spmd

    nc = get_nc()
    in_maps = make_in_maps(template, source)
    res = run_bass_kernel_spmd(nc, in_maps, list(range(N_CORES))).results
    return finalize(res)


# revision 9
# speedup vs baseline: 1.1482x; 1.1482x over previous
"""Chamfer distance loss kernel for Trainium2 (8 NeuronCores).

Problem: template [4, 8192, 3] f32, source [4, 8192, 3] f32 ->
scalar 0.5*(mean_n sqrt(min_m d2) + mean_m sqrt(min_n d2)) over all batches,
d2 = squared euclidean distance, clamped at 0.

Sharding: core c handles batch b = c//2, template half h = c%2
(4096 template rows x all 8192 source points).

Device algorithm (per core):
  e[n, m] = t.s - 0.5||t||^2 - 0.5||s||^2  = -0.5*d2[n, m]
  computed as a K=13 fp16 split-precision matmul (hi/lo decomposition of
  the coordinates and norms, products accumulated in fp32 PSUM) -- full
  fp32-grade accuracy at 1 cycle/row on the PE.
  Row mins:  d2rowmin = max(-2 * max_m e, 0)  -- DVE tensor_scalar max-accum.
  Col mins:  partial max_n e accumulated elementwise (DVE tensor_tensor max),
  final partition/core reduction + sqrt/mean on host (tiny arrays).
"""

import numpy as np

F16 = np.float16
F32 = np.float32

B, N, M, D = 4, 8192, 8192, 3
N_CORES = 8
NSHARD = N // 2          # template rows per core (4096)
NT = NSHARD // 128       # n-tiles per core (32)
MG = M // 2048           # psum groups per n-tile (4)
K = 13                   # augmented contraction dim

_NC_CACHE = {}


def _build_nc():
    import concourse.bacc as bacc
    import concourse.mybir as mybir
    from concourse.tile import TileContext

    f16 = mybir.dt.float16
    f32 = mybir.dt.float32
    Alu = mybir.AluOpType

    nc = bacc.Bacc()
    lhsT = nc.declare_dram_parameter("lhsT", [K, NSHARD], f16, isOutput=False)
    rhs = nc.declare_dram_parameter("rhs", [K, M], f16, isOutput=False)
    rowmax_o = nc.declare_dram_parameter("rowmax", [128, NT], f32, isOutput=True)
    colmax_o = nc.declare_dram_parameter("colmax", [128, M], f16, isOutput=True)

    with TileContext(nc) as tc:
        with (
            tc.tile_pool(name="const", bufs=1) as cpool,
            tc.tile_pool(name="psum", bufs=2, space="PSUM") as ppool,
            tc.tile_pool(name="ebuf", bufs=2) as epool,
        ):
            lhsT_sb = cpool.tile([K, NSHARD], f16)
            rhs_sb = cpool.tile([K, M], f16)
            nc.gpsimd.dma_start(lhsT_sb[:], lhsT[:])
            nc.gpsimd.dma_start(rhs_sb[:], rhs[:])

            cmax = cpool.tile([128, M], f16)
            rowmax = cpool.tile([128, NT], f32)
            scratch = cpool.tile([128, M], f16)
            nc.vector.memset(cmax[:], -60000.0)

            for ti in range(NT):
                e = epool.tile([128, M], f16, tag="e")
                lw = lhsT_sb[:, ti * 128:(ti + 1) * 128]
                for g in range(MG):
                    ps = ppool.tile([128, 2048], f32, tag="ps")
                    for j in range(4):
                        mj = g * 4 + j
                        nc.tensor.matmul(
                            ps[:, j * 512:(j + 1) * 512],
                            lw,
                            rhs_sb[:, mj * 512:(mj + 1) * 512],
                            start=True,
                            stop=True,
                        )
                    nc.scalar.copy(e[:, g * 2048:(g + 1) * 2048], ps[:])
                # row maxes of this n-tile: pairwise fold tree at 2x, then one
                # 1x max-reduce of the 512-wide remainder
                nc.vector.tensor_tensor(
                    scratch[:, 0:4096], e[:, 0:4096], e[:, 4096:8192], Alu.max)
                nc.vector.tensor_tensor(
                    scratch[:, 4096:6144], scratch[:, 0:2048],
                    scratch[:, 2048:4096], Alu.max)
                nc.vector.tensor_tensor(
                    scratch[:, 6144:7168], scratch[:, 4096:5120],
                    scratch[:, 5120:6144], Alu.max)
                nc.vector.tensor_tensor(
                    scratch[:, 7168:7680], scratch[:, 6144:6656],
                    scratch[:, 6656:7168], Alu.max)
                nc.vector.tensor_reduce(
                    rowmax[:, ti:ti + 1], scratch[:, 7168:7680],
                    axis=mybir.AxisListType.X, op=Alu.max)
                # col maxes accumulated across n-tiles (2x mode)
                nc.vector.tensor_tensor(cmax[:], cmax[:], e[:], Alu.max)

            nc.gpsimd.dma_start(rowmax_o[:], rowmax[:])
            nc.gpsimd.dma_start(colmax_o[:], cmax[:])
    return nc


def get_nc():
    if "nc" not in _NC_CACHE:
        nc = _build_nc()
        nc.finalize()
        _NC_CACHE["nc"] = nc
    return _NC_CACHE["nc"]


def _split16(x32):
    """Split fp32 array into (hi, lo) fp16 pair with hi + lo ~= x."""
    hi = x32.astype(F16)
    lo = (x32 - hi.astype(F32)).astype(F16)
    return hi, lo


def _build_lhsT(t):
    """t: [n, 3] f32 template shard -> [13, n] f16 stationary operand."""
    n = t.shape[0]
    th, tl = _split16(t)
    t2 = (t * t).sum(axis=1, dtype=F32)
    u = -0.5 * t2
    uh, ul = _split16(u)
    out = np.empty((K, n), dtype=F16)
    out[0:3] = th.T
    out[3:6] = tl.T
    out[6:9] = th.T
    out[9] = uh
    out[10] = ul
    out[11] = 1.0
    out[12] = 1.0
    return out


def _build_rhs(s):
    """s: [m, 3] f32 source -> [13, m] f16 moving operand."""
    m = s.shape[0]
    sh, sl = _split16(s)
    s2 = (s * s).sum(axis=1, dtype=F32)
    v = -0.5 * s2
    vh, vl = _split16(v)
    out = np.empty((K, m), dtype=F16)
    out[0:3] = sh.T
    out[3:6] = sh.T
    out[6:9] = sl.T
    out[9] = 1.0
    out[10] = 1.0
    out[11] = vh
    out[12] = vl
    return out


def make_in_maps(template, source):
    template = np.asarray(template, dtype=F32)
    source = np.asarray(source, dtype=F32)
    in_maps = []
    for c in range(N_CORES):
        b, h = divmod(c, 2)
        t = template[b, h * NSHARD:(h + 1) * NSHARD]
        s = source[b]
        in_maps.append({"lhsT": _build_lhsT(t), "rhs": _build_rhs(s)})
    return in_maps


def finalize(results):
    """results: list of 8 dicts with 'rowmax' [128, NT] f32, 'colmax' [128, M] f16."""
    row_sqrts = []
    for c in range(N_CORES):
        rm = np.asarray(results[c]["rowmax"], dtype=F32)
        row_sqrts.append(np.sqrt(np.maximum(-2.0 * rm, 0.0), dtype=F32).ravel())
    c01 = np.mean(np.concatenate(row_sqrts), dtype=F32)

    col_sqrts = []
    for b in range(B):
        cm0 = np.asarray(results[2 * b]["colmax"])
        cm1 = np.asarray(results[2 * b + 1]["colmax"])
        cm = np.maximum(cm0, cm1).max(axis=0).astype(F32)  # [M]
        col_sqrts.append(np.sqrt(np.maximum(-2.0 * cm, 0.0), dtype=F32))
    c10 = np.mean(np.concatenate(col_sqrts), dtype=F32)
    return np.float32((c01 + c10) * 0.5)


def kernel(template, source):
    from concourse.bass_utils import run_bass_kernel_spmd

    nc = get_nc()
    in_maps = make_in_maps(template, source)
    res = run_bass_kernel_spmd(nc, in_maps, list(range(N_CORES))).results
    return finalize(res)


# revision 10
# speedup vs baseline: 1.3718x; 1.1948x over previous
"""Chamfer distance loss kernel for Trainium2 (8 NeuronCores).

Problem: template [4, 8192, 3] f32, source [4, 8192, 3] f32 ->
scalar 0.5*(mean_n sqrt(min_m d2) + mean_m sqrt(min_n d2)) over all batches,
d2 = squared euclidean distance, clamped at 0.

Sharding: core c handles batch b = c//2, template half h = c%2
(4096 template rows x all 8192 source points).

Device algorithm (per core):
  e[n, m] = t.s - 0.5||t||^2 - 0.5||s||^2  = -0.5*d2[n, m]
  computed as a K=13 fp16 split-precision matmul (hi/lo decomposition of
  the coordinates and norms, products accumulated in fp32 PSUM) -- full
  fp32-grade accuracy at 1 cycle/row on the PE.
  Row mins:  d2rowmin = max(-2 * max_m e, 0)  -- DVE tensor_scalar max-accum.
  Col mins:  partial max_n e accumulated elementwise (DVE tensor_tensor max),
  final partition/core reduction + sqrt/mean on host (tiny arrays).
"""

import numpy as np

F16 = np.float16
F32 = np.float32

B, N, M, D = 4, 8192, 8192, 3
N_CORES = 8
NSHARD = N // 2          # template rows per core (4096)
NT = NSHARD // 128       # n-tiles per core (32)
MG = M // 2048           # psum groups per n-tile (4)
K = 13                   # augmented contraction dim

_NC_CACHE = {}


def _build_nc():
    import concourse.bacc as bacc
    import concourse.mybir as mybir
    from concourse.tile import TileContext

    f16 = mybir.dt.float16
    f32 = mybir.dt.float32
    Alu = mybir.AluOpType

    nc = bacc.Bacc()
    lhsT = nc.declare_dram_parameter("lhsT", [K, NSHARD], f16, isOutput=False)
    rhs = nc.declare_dram_parameter("rhs", [K, M], f16, isOutput=False)
    rowmax_o = nc.declare_dram_parameter("rowmax", [128, NT], f32, isOutput=True)
    colmax_o = nc.declare_dram_parameter("colmax", [128, M], f16, isOutput=True)

    with TileContext(nc) as tc:
        with (
            tc.tile_pool(name="const", bufs=1) as cpool,
            tc.tile_pool(name="psum", bufs=2, space="PSUM") as ppool,
            tc.tile_pool(name="ebuf", bufs=3) as epool,
        ):
            lhsT_sb = cpool.tile([K, NSHARD], f16)
            rhs_sb = cpool.tile([K, M], f16)
            nc.gpsimd.dma_start(lhsT_sb[:], lhsT[:])
            nc.gpsimd.dma_start(rhs_sb[:], rhs[:])

            cmax = cpool.tile([128, M], f16)
            rowmax = cpool.tile([128, NT], f32)
            scratch = cpool.tile([128, M], f16)
            nc.vector.memset(cmax[:], -60000.0)

            for ti in range(NT):
                e = epool.tile([128, M], f16, tag="e")
                lw = lhsT_sb[:, ti * 128:(ti + 1) * 128]
                for g in range(MG):
                    ps = ppool.tile([128, 2048], f32, tag="ps")
                    for j in range(4):
                        mj = g * 4 + j
                        nc.tensor.matmul(
                            ps[:, j * 512:(j + 1) * 512],
                            lw,
                            rhs_sb[:, mj * 512:(mj + 1) * 512],
                            start=True,
                            stop=True,
                        )
                    nc.scalar.copy(e[:, g * 2048:(g + 1) * 2048], ps[:])
                # row maxes of this n-tile: pairwise fold tree at 2x, then one
                # 1x max-reduce of the 512-wide remainder
                nc.vector.tensor_tensor(
                    scratch[:, 0:4096], e[:, 0:4096], e[:, 4096:8192], Alu.max)
                nc.vector.tensor_tensor(
                    scratch[:, 4096:6144], scratch[:, 0:2048],
                    scratch[:, 2048:4096], Alu.max)
                nc.vector.tensor_tensor(
                    scratch[:, 6144:7168], scratch[:, 4096:5120],
                    scratch[:, 5120:6144], Alu.max)
                nc.vector.tensor_tensor(
                    scratch[:, 7168:7680], scratch[:, 6144:6656],
                    scratch[:, 6656:7168], Alu.max)
                nc.vector.tensor_reduce(
                    rowmax[:, ti:ti + 1], scratch[:, 7168:7680],
                    axis=mybir.AxisListType.X, op=Alu.max)
                # col maxes accumulated across n-tiles (2x mode)
                nc.vector.tensor_tensor(cmax[:], cmax[:], e[:], Alu.max)

            nc.gpsimd.dma_start(rowmax_o[:], rowmax[:])
            nc.gpsimd.dma_start(colmax_o[:], cmax[:])
    return nc


def get_nc():
    if "nc" not in _NC_CACHE:
        nc = _build_nc()
        nc.finalize()
        _NC_CACHE["nc"] = nc
    return _NC_CACHE["nc"]


def _split16(x32):
    """Split fp32 array into (hi, lo) fp16 pair with hi + lo ~= x."""
    hi = x32.astype(F16)
    lo = (x32 - hi.astype(F32)).astype(F16)
    return hi, lo


def _build_lhsT(t):
    """t: [n, 3] f32 template shard -> [13, n] f16 stationary operand."""
    n = t.shape[0]
    th, tl = _split16(t)
    t2 = (t * t).sum(axis=1, dtype=F32)
    u = -0.5 * t2
    uh, ul = _split16(u)
    out = np.empty((K, n), dtype=F16)
    out[0:3] = th.T
    out[3:6] = tl.T
    out[6:9] = th.T
    out[9] = uh
    out[10] = ul
    out[11] = 1.0
    out[12] = 1.0
    return out


def _build_rhs(s):
    """s: [m, 3] f32 source -> [13, m] f16 moving operand."""
    m = s.shape[0]
    sh, sl = _split16(s)
    s2 = (s * s).sum(axis=1, dtype=F32)
    v = -0.5 * s2
    vh, vl = _split16(v)
    out = np.empty((K, m), dtype=F16)
    out[0:3] = sh.T
    out[3:6] = sh.T
    out[6:9] = sl.T
    out[9] = 1.0
    out[10] = 1.0
    out[11] = vh
    out[12] = vl
    return out


def make_in_maps(template, source):
    template = np.asarray(template, dtype=F32)
    source = np.asarray(source, dtype=F32)
    in_maps = []
    for c in range(N_CORES):
        b, h = divmod(c, 2)
        t = template[b, h * NSHARD:(h + 1) * NSHARD]
        s = source[b]
        in_maps.append({"lhsT": _build_lhsT(t), "rhs": _build_rhs(s)})
    return in_maps


def finalize(results):
    """results: list of 8 dicts with 'rowmax' [128, NT] f32, 'colmax' [128, M] f16."""
    row_sqrts = []
    for c in range(N_CORES):
        rm = np.asarray(results[c]["rowmax"], dtype=F32)
        row_sqrts.append(np.sqrt(np.maximum(-2.0 * rm, 0.0), dtype=F32).ravel())
    c01 = np.mean(np.concatenate(row_sqrts), dtype=F32)

    col_sqrts = []
    for b in range(B):
        cm0 = np.asarray(results[2 * b]["colmax"])
        cm1 = np.asarray(results[2 * b + 1]["colmax"])
        cm = np.maximum(cm0, cm1).max(axis=0).astype(F32)  # [M]
        col_sqrts.append(np.sqrt(np.maximum(-2.0 * cm, 0.0), dtype=F32))
    c10 = np.mean(np.concatenate(col_sqrts), dtype=F32)
    return np.float32((c01 + c10) * 0.5)


def kernel(template, source):
    from concourse.bass_utils import run_bass_kernel_spmd

    nc = get_nc()
    in_maps = make_in_maps(template, source)
    res = run_bass_kernel_spmd(nc, in_maps, list(range(N_CORES))).results
    return finalize(res)
